# revision 24
# baseline (speedup 1.0000x reference)
"""Trainium2 Bass kernel for nn_CalibrationError (ECE/MCE over softmax confidences).

Contract: kernel(logits[N,C] f32, labels[N] int64) -> (ece, mce) f32 scalars,
matching reference.py. Internally shards rows across 8 NeuronCores, computes a
15-bin cumulative (count, sum_conf, sum_acc) histogram on-device per core, and
finishes the tiny ECE/MCE arithmetic on host.

Self-contained: hardcodes shapes/sharding; only imports the concourse toolchain.
"""

import sys

if "/opt/trn_rl_repo" not in sys.path:
    sys.path.insert(0, "/opt/trn_rl_repo")

import numpy as np

import concourse.bass as bass
import concourse.bacc as bacc
import concourse.mybir as mybir
from concourse.tile import TileContext
from contextlib import ExitStack

# ---------------------------------------------------------------- constants
P = 128          # SBUF partitions
C = 100          # classes
R = 40           # rows per partition per tile
T = 49           # tiles per core
NCORES = 8
NBINS = 15
ROWS_PER_CORE = P * R * T          # 250_880 (incl. padding)
REAL_ROWS_PER_CORE = 2_000_000 // NCORES  # 250_000
PAD_LOGIT = -10000.0               # exp() underflows to exactly 0.0

f32 = mybir.dt.float32
i32 = mybir.dt.int32
Alu = mybir.AluOpType
Act = mybir.ActivationFunctionType

# Toggle: row-sums via PE identity-chunk matmuls vs DVE reduce. Measured on
# HW: f32 matmuls run as 2 half-speed passes, each reloading the stationary
# weights (~426ns per 40-row chunk) — PE sums are 3x slower than DVE reduce.
USE_PE_SUM = False
# Rows-per-partition whose pack step runs on GPSIMD instead of DVE (0..R).
# Walrus rejects InstTensorScalarPtr on the Pool engine (no Q7 ucode), so
# this must stay 0 unless the pack is reformulated as plain tensor_tensor.
POOL_ROWS = 0


def build_nc(p=P, c=C, r=R, t=T):
    """Build the per-core Bass module (SPMD: same program on all cores).

    Uses Bacc so finalize() runs generate_event_semaphores() — walrus allows
    at most one sync wait per engine instruction; Bacc splits the excess.
    """
    nc = bacc.Bacc()

    x = nc.declare_dram_parameter("x", [t * p * r, c], f32, isOutput=False)
    lab = nc.declare_dram_parameter("lab", [t, p, r], i32, isOutput=False)
    rev = nc.declare_dram_parameter("rev", [p, r * c], i32, isOutput=False)
    ident = nc.declare_dram_parameter("ident", [p, p], f32, isOutput=False)
    out = nc.declare_dram_parameter("out", [NBINS, 3], f32, isOutput=True)

    xv = x[:, :].rearrange("(t p r) c -> t p (r c)", t=t, p=p, r=r)

    with TileContext(nc) as tc, ExitStack() as ctx:
        consts = ctx.enter_context(tc.tile_pool(name="consts", bufs=1))
        work = ctx.enter_context(tc.tile_pool(name="work", bufs=2))
        small = ctx.enter_context(tc.tile_pool(name="small", bufs=3))
        psum = ctx.enter_context(tc.tile_pool(name="psum", bufs=2, space="PSUM"))

        rev_t = consts.tile([p, r * c], i32, tag="rev_t")
        nc.sync.dma_start(out=rev_t[:], in_=rev[:, :])
        ident_t = consts.tile([p, p], f32, tag="ident_t")
        nc.sync.dma_start(out=ident_t[:], in_=ident[:, :])
        mask_hi = consts.tile([p, 1], i32, tag="mask_hi")  # ~127
        nc.vector.memset(mask_hi[:], -128)
        mask_lo = consts.tile([p, 1], i32, tag="mask_lo")  # 127
        nc.vector.memset(mask_lo[:], 127)
        hist = consts.tile([NBINS, 3], f32, tag="hist")
        nc.vector.memset(hist[:], 0.0)
        zeros_i = consts.tile([p, r], i32, tag="zeros_i")
        nc.vector.memset(zeros_i[:], 0)

        # Engine warmups: absorb the const-tile DMA waits on throwaway ops so
        # first-iteration instructions carry few sync waits (walrus limits
        # the wait-command count per instruction).
        warm = psum.tile([p, 1], f32, tag="warm")
        nc.tensor.matmul(
            warm[:], lhsT=ident_t[:], rhs=ident_t[:, 0:1], start=True, stop=True
        )
        scr_v = consts.tile([p, 1], i32, tag="scr_v")
        nc.vector.tensor_copy(out=scr_v[:], in_=rev_t[:, 0:1])
        scr_m = consts.tile([p, 1], i32, tag="scr_m")
        nc.vector.tensor_tensor(
            out=scr_m[:], in0=mask_hi[:], in1=mask_lo[:], op=Alu.bitwise_and
        )
        nc.vector.tensor_tensor(
            out=scr_m[:], in0=scr_m[:], in1=zeros_i[:, 0:1], op=Alu.bitwise_or
        )
        scr_g = consts.tile([p, 1], i32, tag="scr_g")
        nc.gpsimd.tensor_copy(out=scr_g[:], in_=rev_t[:, 0:1])
        scr_g2 = consts.tile([p, 1], i32, tag="scr_g2")
        nc.gpsimd.tensor_copy(out=scr_g2[:], in_=mask_hi[:])

        pend = []

        def _emit_hist(gv):
            g3p, v3p = gv
            ph = psum.tile([NBINS, 3], f32, tag="ph")
            for rr in range(r):
                nc.tensor.matmul(
                    ph[:],
                    lhsT=g3p[:, :, rr],
                    rhs=v3p[:, :, rr],
                    start=(rr == 0),
                    stop=(rr == r - 1),
                )
            nc.vector.tensor_tensor(
                out=hist[:], in0=hist[:], in1=ph[:], op=Alu.add
            )

        for it in range(t):
            xt = work.tile([p, r * c], f32, tag="xt")
            nc.sync.dma_start(out=xt[:], in_=xv[it])
            labt = work.tile([p, r], i32, tag="labt")
            nc.sync.dma_start(out=labt[:], in_=lab[it, :, :])

            # e = exp(x)  (no max-subtraction needed: |x| < 90)
            et = work.tile([p, r * c], f32, tag="et")
            nc.scalar.activation(out=et[:], in_=xt[:], func=Act.Exp)
            e3 = et[:].rearrange("p (r c) -> p r c", r=r)

            # pack value+index: pk = (bits(e) & ~127) | (127 - class).
            # Optionally split along rows between GPSIMD (idle) and DVE.
            ra = min(POOL_ROWS, r)
            pm = small.tile([p, r], f32, tag="pm")
            if ra > 0:
                pka = work.tile([p, ra * c], i32, tag="pka")
                nc.gpsimd.scalar_tensor_tensor(
                    out=pka[:],
                    in0=et[:, : ra * c].bitcast(i32),
                    scalar=mask_hi[:],
                    in1=rev_t[:, : ra * c],
                    op0=Alu.bitwise_and,
                    op1=Alu.bitwise_or,
                )
            if ra < r:
                pk = work.tile([p, (r - ra) * c], i32, tag="pk")
                nc.vector.scalar_tensor_tensor(
                    out=pk[:],
                    in0=et[:, ra * c :].bitcast(i32),
                    scalar=mask_hi[:],
                    in1=rev_t[:, ra * c :],
                    op0=Alu.bitwise_and,
                    op1=Alu.bitwise_or,
                )
            # grouped argmax+max: pm[p, r] = max over classes. The DVE ALU is
            # fp32-internal, so reduce the packed bits AS float32: packed
            # values are positive normal floats, where fp32 ordering equals
            # bit ordering — the max is exact and index bits survive.
            if ra > 0:
                nc.vector.tensor_reduce(
                    out=pm[:, :ra],
                    in_=pka[:].bitcast(f32).rearrange("p (r c) -> p r c", r=ra),
                    axis=mybir.AxisListType.X,
                    op=Alu.max,
                )
            if ra < r:
                nc.vector.tensor_reduce(
                    out=pm[:, ra:],
                    in_=pk[:].bitcast(f32).rearrange("p (r c) -> p r c", r=r - ra),
                    axis=mybir.AxisListType.X,
                    op=Alu.max,
                )

            # row sums s[p, r] = sum_c e[p, r, c]
            if USE_PE_SUM:
                ps = psum.tile([p, r], f32, tag="ps")
                for cc in range(c):
                    nc.tensor.matmul(
                        ps[:],
                        lhsT=ident_t[:],
                        rhs=e3[:, :, cc],
                        start=(cc == 0),
                        stop=(cc == c - 1),
                    )
                s_sb = small.tile([p, r], f32, tag="s_sb")
                nc.vector.tensor_copy(out=s_sb[:], in_=ps[:])
            else:
                s_sb = small.tile([p, r], f32, tag="s_sb")
                nc.vector.tensor_reduce(
                    out=s_sb[:], in_=e3, axis=mybir.AxisListType.X, op=Alu.add
                )

            # clamp away zeros (pad rows) then reciprocal
            nc.vector.tensor_scalar_max(s_sb[:], s_sb[:], 1e-30)
            rs = small.tile([p, r], f32, tag="rs")
            nc.vector.reciprocal(out=rs[:], in_=s_sb[:])

            # vals = [conf, acc, ones] laid out [p, 3, r]
            vals = small.tile([p, 3 * r], f32, tag="vals")
            v3 = vals[:].rearrange("p (k r) -> p k r", k=3)

            # me = float(pm & ~127); conf = me * (1/s)
            meb = small.tile([p, r], i32, tag="meb")
            nc.vector.scalar_tensor_tensor(
                out=meb[:], in0=pm[:].bitcast(i32), scalar=mask_hi[:],
                in1=zeros_i[:], op0=Alu.bitwise_and, op1=Alu.bitwise_or,
            )
            nc.vector.tensor_tensor(
                out=v3[:, 0, :], in0=meb[:].bitcast(f32), in1=rs[:], op=Alu.mult
            )
            # acc = (pm & 127) == (127 - label)
            jrev = small.tile([p, r], i32, tag="jrev")
            nc.vector.scalar_tensor_tensor(
                out=jrev[:], in0=pm[:].bitcast(i32), scalar=mask_lo[:],
                in1=zeros_i[:], op0=Alu.bitwise_and, op1=Alu.bitwise_or,
            )
            nc.vector.tensor_tensor(
                out=v3[:, 1, :], in0=jrev[:], in1=labt[:], op=Alu.is_equal
            )
            nc.vector.memset(v3[:, 2, :], 1.0)

            # ge[p, b, r] = conf > b/15   (strict: pad rows have conf == 0.0).
            # Runs on GPSIMD (float compare, imm scalar) to offload the DVE,
            # which is the bottleneck engine.
            ge = small.tile([p, NBINS * r], f32, tag="ge")
            g3 = ge[:].rearrange("p (b r) -> p b r", b=NBINS)
            for b in range(NBINS):
                nc.gpsimd.tensor_single_scalar(
                    out=g3[:, b, :], in_=v3[:, 0, :], scalar=float(b) / NBINS,
                    op=Alu.is_gt,
                )

            # cumulative histogram: out[b, k] += sum_rows ge_b * vals_k.
            # Emitted one tile late so PE never stalls waiting for this
            # tile's DVE chain (keeps chunk-sum matmuls flowing).
            pend.append((g3, v3))
            if len(pend) > 1:
                _emit_hist(pend.pop(0))
        _emit_hist(pend.pop(0))

        nc.sync.dma_start(out=out[:, :], in_=hist[:])

    nc.finalize()
    return nc


# ---------------------------------------------------------------- host side

def _prep_core_inputs(logits, labels, core):
    """Build the per-core input dict (padded, tile-layout labels)."""
    lo = core * REAL_ROWS_PER_CORE
    hi = lo + REAL_ROWS_PER_CORE
    x = np.full((ROWS_PER_CORE, C), PAD_LOGIT, dtype=np.float32)
    x[: REAL_ROWS_PER_CORE] = logits[lo:hi]
    lab = np.zeros(ROWS_PER_CORE, dtype=np.int32)
    lab[: REAL_ROWS_PER_CORE] = labels[lo:hi].astype(np.int32)
    labrev = (127 - lab).reshape(T, P, R)
    return {"x": x, "lab": labrev}


def _shared_inputs():
    rev = np.broadcast_to(
        (127 - np.arange(C, dtype=np.int32))[None, None, :], (P, R, C)
    ).reshape(P, R * C).copy()
    ident = np.eye(P, dtype=np.float32)
    return {"rev": rev, "ident": ident}


def _finish(hists):
    """hists: list of [15, 3] cumulative-threshold sums -> (ece, mce)."""
    cum = np.zeros((NBINS + 1, 3), dtype=np.float64)
    for h in hists:
        cum[:NBINS] += h.astype(np.float64)
    per_bin = cum[:NBINS] - cum[1:]  # [15, 3]: sum_conf, sum_acc, count
    sum_conf, sum_acc, counts = per_bin[:, 0], per_bin[:, 1], per_bin[:, 2]
    nonempty = counts > 0
    safe = np.where(nonempty, counts, 1.0)
    gap = np.abs(sum_conf / safe - sum_acc / safe)
    n_total = float(2_000_000)
    ece = np.sum(np.where(nonempty, gap * counts / n_total, 0.0))
    mce = np.max(np.where(nonempty, gap, -np.inf)) if nonempty.any() else 1.0
    return np.float32(ece), np.float32(mce)


_NC_CACHE = {}


def kernel(logits, labels):
    from concourse.bass_utils import run_bass_kernel_spmd

    logits = np.asarray(logits, dtype=np.float32)
    labels = np.asarray(labels)

    if "nc" not in _NC_CACHE:
        _NC_CACHE["nc"] = build_nc()
    nc = _NC_CACHE["nc"]

    shared = _shared_inputs()
    in_maps = [
        {**_prep_core_inputs(logits, labels, core), **shared}
        for core in range(NCORES)
    ]
    res = run_bass_kernel_spmd(nc, in_maps, list(range(NCORES)))
    hists = [res.results[i]["out"] for i in range(NCORES)]
    return _finish(hists)


# revision 28
# speedup vs baseline: 1.2783x; 1.2783x over previous
"""Trainium2 Bass kernel for nn_CalibrationError (ECE/MCE over softmax confidences).

Contract: kernel(logits[N,C] f32, labels[N] int64) -> (ece, mce) f32 scalars,
matching reference.py. Internally shards rows across 8 NeuronCores, computes a
15-bin cumulative (count, sum_conf, sum_acc) histogram on-device per core, and
finishes the tiny ECE/MCE arithmetic on host.

Self-contained: hardcodes shapes/sharding; only imports the concourse toolchain.
"""

import sys

if "/opt/trn_rl_repo" not in sys.path:
    sys.path.insert(0, "/opt/trn_rl_repo")

import numpy as np

import concourse.bass as bass
import concourse.bacc as bacc
import concourse.mybir as mybir
from concourse.tile import TileContext
from contextlib import ExitStack

# ---------------------------------------------------------------- constants
P = 128          # SBUF partitions
C = 100          # classes
R = 40           # rows per partition per tile
T = 49           # tiles per core
NCORES = 8
NBINS = 15
ROWS_PER_CORE = P * R * T          # 250_880 (incl. padding)
REAL_ROWS_PER_CORE = 2_000_000 // NCORES  # 250_000
PAD_LOGIT = -10000.0               # exp() underflows to exactly 0.0

f32 = mybir.dt.float32
i32 = mybir.dt.int32
Alu = mybir.AluOpType
Act = mybir.ActivationFunctionType

# Row-sum split: PE sums the first PE_SUM_ROWS rows (batched identity
# matmuls, G columns per matmul so the f32 weight reload amortizes over a
# 500-element moving block), DVE reduces the rest. f32 matmuls run as two
# half-speed passes each reloading weights, so PE rows cost ~334ns/row vs
# DVE's 104ns/row — but PE is otherwise idle while DVE is the bottleneck.
PE_SUM_ROWS = 25
PE_SUM_G = 20  # PE_SUM_ROWS * PE_SUM_G must be <= 512 (max moving free dim)
# Rows-per-partition whose pack step runs on GPSIMD instead of DVE (0..R).
# Walrus rejects InstTensorScalarPtr on the Pool engine (no Q7 ucode), so
# this must stay 0 unless the pack is reformulated as plain tensor_tensor.
POOL_ROWS = 0


def build_nc(p=P, c=C, r=R, t=T):
    """Build the per-core Bass module (SPMD: same program on all cores).

    Uses Bacc so finalize() runs generate_event_semaphores() — walrus allows
    at most one sync wait per engine instruction; Bacc splits the excess.
    """
    nc = bacc.Bacc()

    x = nc.declare_dram_parameter("x", [t * p * r, c], f32, isOutput=False)
    lab = nc.declare_dram_parameter("lab", [t, p, r], i32, isOutput=False)
    rev = nc.declare_dram_parameter("rev", [p, r * c], i32, isOutput=False)
    ident = nc.declare_dram_parameter("ident", [p, p], f32, isOutput=False)
    out = nc.declare_dram_parameter("out", [NBINS, 3], f32, isOutput=True)

    xv = x[:, :].rearrange("(t p r) c -> t p (r c)", t=t, p=p, r=r)

    with TileContext(nc) as tc, ExitStack() as ctx:
        consts = ctx.enter_context(tc.tile_pool(name="consts", bufs=1))
        work = ctx.enter_context(tc.tile_pool(name="work", bufs=2))
        small = ctx.enter_context(tc.tile_pool(name="small", bufs=3))
        psum = ctx.enter_context(tc.tile_pool(name="psum", bufs=2, space="PSUM"))

        rev_t = consts.tile([p, r * c], i32, tag="rev_t")
        nc.sync.dma_start(out=rev_t[:], in_=rev[:, :])
        ident_t = consts.tile([p, p], f32, tag="ident_t")
        nc.sync.dma_start(out=ident_t[:], in_=ident[:, :])
        mask_hi = consts.tile([p, 1], i32, tag="mask_hi")  # ~127
        nc.vector.memset(mask_hi[:], -128)
        mask_lo = consts.tile([p, 1], i32, tag="mask_lo")  # 127
        nc.vector.memset(mask_lo[:], 127)
        hist = consts.tile([NBINS, 3], f32, tag="hist")
        nc.vector.memset(hist[:], 0.0)
        zeros_i = consts.tile([p, r], i32, tag="zeros_i")
        nc.vector.memset(zeros_i[:], 0)

        # Engine warmups: absorb the const-tile DMA waits on throwaway ops so
        # first-iteration instructions carry few sync waits (walrus limits
        # the wait-command count per instruction).
        warm = psum.tile([p, 1], f32, tag="warm")
        nc.tensor.matmul(
            warm[:], lhsT=ident_t[:], rhs=ident_t[:, 0:1], start=True, stop=True
        )
        scr_v = consts.tile([p, 1], i32, tag="scr_v")
        nc.vector.tensor_copy(out=scr_v[:], in_=rev_t[:, 0:1])
        scr_m = consts.tile([p, 1], i32, tag="scr_m")
        nc.vector.tensor_tensor(
            out=scr_m[:], in0=mask_hi[:], in1=mask_lo[:], op=Alu.bitwise_and
        )
        nc.vector.tensor_tensor(
            out=scr_m[:], in0=scr_m[:], in1=zeros_i[:, 0:1], op=Alu.bitwise_or
        )
        if POOL_ROWS > 0:
            scr_g = consts.tile([p, 1], i32, tag="scr_g")
            nc.gpsimd.tensor_copy(out=scr_g[:], in_=rev_t[:, 0:1])
            scr_g2 = consts.tile([p, 1], i32, tag="scr_g2")
            nc.gpsimd.tensor_copy(out=scr_g2[:], in_=mask_hi[:])

        pend = []

        def _emit_hist(gv):
            g3p, v3p = gv
            ph = psum.tile([NBINS, 3], f32, tag="ph")
            for rr in range(r):
                nc.tensor.matmul(
                    ph[:],
                    lhsT=g3p[:, :, rr],
                    rhs=v3p[:, :, rr],
                    start=(rr == 0),
                    stop=(rr == r - 1),
                )
            nc.vector.tensor_tensor(
                out=hist[:], in0=hist[:], in1=ph[:], op=Alu.add
            )

        for it in range(t):
            xt = work.tile([p, r * c], f32, tag="xt")
            nc.sync.dma_start(out=xt[:], in_=xv[it])
            labt = work.tile([p, r], i32, tag="labt")
            nc.sync.dma_start(out=labt[:], in_=lab[it, :, :])

            # e = exp(x)  (no max-subtraction needed: |x| < 90)
            et = work.tile([p, r * c], f32, tag="et")
            nc.scalar.activation(out=et[:], in_=xt[:], func=Act.Exp)
            e3 = et[:].rearrange("p (r c) -> p r c", r=r)

            # pack value+index: pk = (bits(e) & ~127) | (127 - class).
            # Optionally split along rows between GPSIMD (idle) and DVE.
            ra = min(POOL_ROWS, r)
            pm = small.tile([p, r], f32, tag="pm")
            if ra > 0:
                pka = work.tile([p, ra * c], i32, tag="pka")
                nc.gpsimd.scalar_tensor_tensor(
                    out=pka[:],
                    in0=et[:, : ra * c].bitcast(i32),
                    scalar=mask_hi[:],
                    in1=rev_t[:, : ra * c],
                    op0=Alu.bitwise_and,
                    op1=Alu.bitwise_or,
                )
            if ra < r:
                pk = work.tile([p, (r - ra) * c], i32, tag="pk")
                nc.vector.scalar_tensor_tensor(
                    out=pk[:],
                    in0=et[:, ra * c :].bitcast(i32),
                    scalar=mask_hi[:],
                    in1=rev_t[:, ra * c :],
                    op0=Alu.bitwise_and,
                    op1=Alu.bitwise_or,
                )
            # grouped argmax+max: pm[p, r] = max over classes. The DVE ALU is
            # fp32-internal, so reduce the packed bits AS float32: packed
            # values are positive normal floats, where fp32 ordering equals
            # bit ordering — the max is exact and index bits survive.
            if ra > 0:
                nc.vector.tensor_reduce(
                    out=pm[:, :ra],
                    in_=pka[:].bitcast(f32).rearrange("p (r c) -> p r c", r=ra),
                    axis=mybir.AxisListType.X,
                    op=Alu.max,
                )
            if ra < r:
                nc.vector.tensor_reduce(
                    out=pm[:, ra:],
                    in_=pk[:].bitcast(f32).rearrange("p (r c) -> p r c", r=r - ra),
                    axis=mybir.AxisListType.X,
                    op=Alu.max,
                )

            # row sums s[p, r] = sum_c e[p, r, c]. Hybrid split: PE sums the
            # first PE_ROWS rows via batched identity matmuls (partial sums
            # per G-column group accumulate in PSUM; DVE combines the G
            # partials), DVE reduces the rest. Balances the two engines —
            # DVE is otherwise the bottleneck.
            s_sb = small.tile([p, r], f32, tag="s_sb")
            gp, gw = min(PE_SUM_ROWS, r - 1), PE_SUM_G
            if gp > 0:
                nchunk = (c + gw - 1) // gw
                pss = psum.tile([p, gp * gw], f32, tag="pss")
                for k in range(nchunk):
                    c0 = k * gw
                    nc.tensor.matmul(
                        pss[:],
                        lhsT=ident_t[:],
                        rhs=e3[:, 0:gp, c0 : c0 + gw],
                        start=(k == 0),
                        stop=(k == nchunk - 1),
                    )
                nc.vector.tensor_reduce(
                    out=s_sb[:, 0:gp],
                    in_=pss[:].rearrange("p (r g) -> p r g", r=gp),
                    axis=mybir.AxisListType.X,
                    op=Alu.add,
                )
            if gp < r:
                nc.vector.tensor_reduce(
                    out=s_sb[:, gp:],
                    in_=e3[:, gp:r, :],
                    axis=mybir.AxisListType.X,
                    op=Alu.add,
                )

            # clamp away zeros (pad rows) then reciprocal
            nc.vector.tensor_scalar_max(s_sb[:], s_sb[:], 1e-30)
            rs = small.tile([p, r], f32, tag="rs")
            nc.vector.reciprocal(out=rs[:], in_=s_sb[:])

            # vals = [conf, acc, ones] laid out [p, 3, r]
            vals = small.tile([p, 3 * r], f32, tag="vals")
            v3 = vals[:].rearrange("p (k r) -> p k r", k=3)

            # me = float(pm & ~127); conf = me * (1/s)
            meb = small.tile([p, r], i32, tag="meb")
            nc.vector.scalar_tensor_tensor(
                out=meb[:], in0=pm[:].bitcast(i32), scalar=mask_hi[:],
                in1=zeros_i[:], op0=Alu.bitwise_and, op1=Alu.bitwise_or,
            )
            nc.vector.tensor_tensor(
                out=v3[:, 0, :], in0=meb[:].bitcast(f32), in1=rs[:], op=Alu.mult
            )
            # acc = (pm & 127) == (127 - label)
            jrev = small.tile([p, r], i32, tag="jrev")
            nc.vector.scalar_tensor_tensor(
                out=jrev[:], in0=pm[:].bitcast(i32), scalar=mask_lo[:],
                in1=zeros_i[:], op0=Alu.bitwise_and, op1=Alu.bitwise_or,
            )
            nc.vector.tensor_tensor(
                out=v3[:, 1, :], in0=jrev[:], in1=labt[:], op=Alu.is_equal
            )
            nc.vector.memset(v3[:, 2, :], 1.0)

            # ge[p, b, r] = conf > b/15   (strict: pad rows have conf == 0.0)
            ge = small.tile([p, NBINS * r], f32, tag="ge")
            g3 = ge[:].rearrange("p (b r) -> p b r", b=NBINS)
            for b in range(NBINS):
                nc.vector.tensor_single_scalar(
                    out=g3[:, b, :], in_=v3[:, 0, :], scalar=float(b) / NBINS,
                    op=Alu.is_gt,
                )

            # cumulative histogram: out[b, k] += sum_rows ge_b * vals_k.
            # Emitted one tile late so PE never stalls waiting for this
            # tile's DVE chain (keeps chunk-sum matmuls flowing).
            pend.append((g3, v3))
            if len(pend) > 1:
                _emit_hist(pend.pop(0))
        _emit_hist(pend.pop(0))

        nc.sync.dma_start(out=out[:, :], in_=hist[:])

    nc.finalize()
    return nc


# ---------------------------------------------------------------- host side

def _prep_core_inputs(logits, labels, core):
    """Build the per-core input dict (padded, tile-layout labels)."""
    lo = core * REAL_ROWS_PER_CORE
    hi = lo + REAL_ROWS_PER_CORE
    x = np.full((ROWS_PER_CORE, C), PAD_LOGIT, dtype=np.float32)
    x[: REAL_ROWS_PER_CORE] = logits[lo:hi]
    lab = np.zeros(ROWS_PER_CORE, dtype=np.int32)
    lab[: REAL_ROWS_PER_CORE] = labels[lo:hi].astype(np.int32)
    labrev = (127 - lab).reshape(T, P, R)
    return {"x": x, "lab": labrev}


def _shared_inputs():
    rev = np.broadcast_to(
        (127 - np.arange(C, dtype=np.int32))[None, None, :], (P, R, C)
    ).reshape(P, R * C).copy()
    ident = np.eye(P, dtype=np.float32)
    return {"rev": rev, "ident": ident}


def _finish(hists):
    """hists: list of [15, 3] cumulative-threshold sums -> (ece, mce)."""
    cum = np.zeros((NBINS + 1, 3), dtype=np.float64)
    for h in hists:
        cum[:NBINS] += h.astype(np.float64)
    per_bin = cum[:NBINS] - cum[1:]  # [15, 3]: sum_conf, sum_acc, count
    sum_conf, sum_acc, counts = per_bin[:, 0], per_bin[:, 1], per_bin[:, 2]
    nonempty = counts > 0
    safe = np.where(nonempty, counts, 1.0)
    gap = np.abs(sum_conf / safe - sum_acc / safe)
    n_total = float(2_000_000)
    ece = np.sum(np.where(nonempty, gap * counts / n_total, 0.0))
    mce = np.max(np.where(nonempty, gap, -np.inf)) if nonempty.any() else 1.0
    return np.float32(ece), np.float32(mce)


_NC_CACHE = {}


def kernel(logits, labels):
    from concourse.bass_utils import run_bass_kernel_spmd

    logits = np.asarray(logits, dtype=np.float32)
    labels = np.asarray(labels)

    if "nc" not in _NC_CACHE:
        _NC_CACHE["nc"] = build_nc()
    nc = _NC_CACHE["nc"]

    shared = _shared_inputs()
    in_maps = [
        {**_prep_core_inputs(logits, labels, core), **shared}
        for core in range(NCORES)
    ]
    res = run_bass_kernel_spmd(nc, in_maps, list(range(NCORES)))
    hists = [res.results[i]["out"] for i in range(NCORES)]
    return _finish(hists)


# revision 32
# speedup vs baseline: 1.2929x; 1.0114x over previous
"""Trainium2 Bass kernel for nn_CalibrationError (ECE/MCE over softmax confidences).

Contract: kernel(logits[N,C] f32, labels[N] int64) -> (ece, mce) f32 scalars,
matching reference.py. Internally shards rows across 8 NeuronCores, computes a
15-bin cumulative (count, sum_conf, sum_acc) histogram on-device per core, and
finishes the tiny ECE/MCE arithmetic on host.

Self-contained: hardcodes shapes/sharding; only imports the concourse toolchain.
"""

import sys

if "/opt/trn_rl_repo" not in sys.path:
    sys.path.insert(0, "/opt/trn_rl_repo")

import numpy as np

import concourse.bass as bass
import concourse.bacc as bacc
import concourse.mybir as mybir
from concourse.tile import TileContext
from contextlib import ExitStack

# ---------------------------------------------------------------- constants
P = 128          # SBUF partitions
C = 100          # classes
R = 56           # rows per partition per tile
T = 35           # tiles per core
NCORES = 8
NBINS = 15
ROWS_PER_CORE = P * R * T          # 250_880 (incl. padding)
REAL_ROWS_PER_CORE = 2_000_000 // NCORES  # 250_000
PAD_LOGIT = -10000.0               # exp() underflows to exactly 0.0

f32 = mybir.dt.float32
i32 = mybir.dt.int32
Alu = mybir.AluOpType
Act = mybir.ActivationFunctionType

# Row-sum split: PE sums the first PE_SUM_ROWS rows (batched identity
# matmuls, G columns per matmul so the f32 weight reload amortizes over a
# 500-element moving block), DVE reduces the rest. f32 matmuls run as two
# half-speed passes each reloading weights, so PE rows cost ~334ns/row vs
# DVE's 104ns/row — but PE is otherwise idle while DVE is the bottleneck.
PE_SUM_ROWS = 32
PE_SUM_G = 16  # PE_SUM_ROWS * PE_SUM_G must be <= 512 (max moving free dim)
# PE covers only the first (C // PE_SUM_G) * PE_SUM_G columns; DVE reduces
# the remainder columns for those rows and adds them in.
# Rows-per-partition whose pack step runs on GPSIMD instead of DVE (0..R).
# Walrus rejects InstTensorScalarPtr on the Pool engine (no Q7 ucode), so
# this must stay 0 unless the pack is reformulated as plain tensor_tensor.
POOL_ROWS = 0


def build_nc(p=P, c=C, r=R, t=T):
    """Build the per-core Bass module (SPMD: same program on all cores).

    Uses Bacc so finalize() runs generate_event_semaphores() — walrus allows
    at most one sync wait per engine instruction; Bacc splits the excess.
    """
    nc = bacc.Bacc()

    x = nc.declare_dram_parameter("x", [t * p * r, c], f32, isOutput=False)
    lab = nc.declare_dram_parameter("lab", [t, p, r], i32, isOutput=False)
    rev = nc.declare_dram_parameter("rev", [p, r * c], i32, isOutput=False)
    ident = nc.declare_dram_parameter("ident", [p, p], f32, isOutput=False)
    out = nc.declare_dram_parameter("out", [NBINS, 3], f32, isOutput=True)

    xv = x[:, :].rearrange("(t p r) c -> t p (r c)", t=t, p=p, r=r)

    with TileContext(nc) as tc, ExitStack() as ctx:
        consts = ctx.enter_context(tc.tile_pool(name="consts", bufs=1))
        work = ctx.enter_context(tc.tile_pool(name="work", bufs=2))
        small = ctx.enter_context(tc.tile_pool(name="small", bufs=3))
        psum = ctx.enter_context(tc.tile_pool(name="psum", bufs=2, space="PSUM"))

        rev_t = consts.tile([p, r * c], i32, tag="rev_t")
        nc.sync.dma_start(out=rev_t[:], in_=rev[:, :])
        ident_t = consts.tile([p, p], f32, tag="ident_t")
        nc.sync.dma_start(out=ident_t[:], in_=ident[:, :])
        mask_hi = consts.tile([p, 1], i32, tag="mask_hi")  # ~127
        nc.vector.memset(mask_hi[:], -128)
        mask_lo = consts.tile([p, 1], i32, tag="mask_lo")  # 127
        nc.vector.memset(mask_lo[:], 127)
        hist = consts.tile([NBINS, 3], f32, tag="hist")
        nc.vector.memset(hist[:], 0.0)
        zeros_i = consts.tile([p, r], i32, tag="zeros_i")
        nc.vector.memset(zeros_i[:], 0)

        # Engine warmups: absorb the const-tile DMA waits on throwaway ops so
        # first-iteration instructions carry few sync waits (walrus limits
        # the wait-command count per instruction).
        warm = psum.tile([p, 1], f32, tag="warm")
        nc.tensor.matmul(
            warm[:], lhsT=ident_t[:], rhs=ident_t[:, 0:1], start=True, stop=True
        )
        scr_v = consts.tile([p, 1], i32, tag="scr_v")
        nc.vector.tensor_copy(out=scr_v[:], in_=rev_t[:, 0:1])
        scr_m = consts.tile([p, 1], i32, tag="scr_m")
        nc.vector.tensor_tensor(
            out=scr_m[:], in0=mask_hi[:], in1=mask_lo[:], op=Alu.bitwise_and
        )
        nc.vector.tensor_tensor(
            out=scr_m[:], in0=scr_m[:], in1=zeros_i[:, 0:1], op=Alu.bitwise_or
        )
        if POOL_ROWS > 0:
            scr_g = consts.tile([p, 1], i32, tag="scr_g")
            nc.gpsimd.tensor_copy(out=scr_g[:], in_=rev_t[:, 0:1])
            scr_g2 = consts.tile([p, 1], i32, tag="scr_g2")
            nc.gpsimd.tensor_copy(out=scr_g2[:], in_=mask_hi[:])

        pend = []

        def _emit_hist(gv):
            g3p, v3p = gv
            ph = psum.tile([NBINS, 3], f32, tag="ph")
            for rr in range(r):
                nc.tensor.matmul(
                    ph[:],
                    lhsT=g3p[:, :, rr],
                    rhs=v3p[:, :, rr],
                    start=(rr == 0),
                    stop=(rr == r - 1),
                )
            nc.vector.tensor_tensor(
                out=hist[:], in0=hist[:], in1=ph[:], op=Alu.add
            )

        for it in range(t):
            xt = work.tile([p, r * c], f32, tag="xt")
            nc.sync.dma_start(out=xt[:], in_=xv[it])
            labt = work.tile([p, r], i32, tag="labt")
            nc.sync.dma_start(out=labt[:], in_=lab[it, :, :])

            # e = exp(x)  (no max-subtraction needed: |x| < 90)
            et = work.tile([p, r * c], f32, tag="et")
            nc.scalar.activation(out=et[:], in_=xt[:], func=Act.Exp)
            e3 = et[:].rearrange("p (r c) -> p r c", r=r)

            # pack value+index: pk = (bits(e) & ~127) | (127 - class).
            # Optionally split along rows between GPSIMD (idle) and DVE.
            ra = min(POOL_ROWS, r)
            pm = small.tile([p, r], f32, tag="pm")
            if ra > 0:
                pka = work.tile([p, ra * c], i32, tag="pka")
                nc.gpsimd.scalar_tensor_tensor(
                    out=pka[:],
                    in0=et[:, : ra * c].bitcast(i32),
                    scalar=mask_hi[:],
                    in1=rev_t[:, : ra * c],
                    op0=Alu.bitwise_and,
                    op1=Alu.bitwise_or,
                )
            if ra < r:
                # bufs=1: pack and reduce are both DVE (sequential), so no
                # cross-tile overlap is lost — and it keeps R=56 within SBUF.
                pk = work.tile([p, (r - ra) * c], i32, tag="pk", bufs=1)
                nc.vector.scalar_tensor_tensor(
                    out=pk[:],
                    in0=et[:, ra * c :].bitcast(i32),
                    scalar=mask_hi[:],
                    in1=rev_t[:, ra * c :],
                    op0=Alu.bitwise_and,
                    op1=Alu.bitwise_or,
                )
            # grouped argmax+max: pm[p, r] = max over classes. The DVE ALU is
            # fp32-internal, so reduce the packed bits AS float32: packed
            # values are positive normal floats, where fp32 ordering equals
            # bit ordering — the max is exact and index bits survive.
            if ra > 0:
                nc.vector.tensor_reduce(
                    out=pm[:, :ra],
                    in_=pka[:].bitcast(f32).rearrange("p (r c) -> p r c", r=ra),
                    axis=mybir.AxisListType.X,
                    op=Alu.max,
                )
            if ra < r:
                nc.vector.tensor_reduce(
                    out=pm[:, ra:],
                    in_=pk[:].bitcast(f32).rearrange("p (r c) -> p r c", r=r - ra),
                    axis=mybir.AxisListType.X,
                    op=Alu.max,
                )

            # row sums s[p, r] = sum_c e[p, r, c]. Hybrid split: PE sums the
            # first PE_ROWS rows via batched identity matmuls (partial sums
            # per G-column group accumulate in PSUM; DVE combines the G
            # partials), DVE reduces the rest. Balances the two engines —
            # DVE is otherwise the bottleneck.
            s_sb = small.tile([p, r], f32, tag="s_sb")
            gp, gw = min(PE_SUM_ROWS, r - 1), PE_SUM_G
            nfull = c // gw          # full-width PE chunks
            crem = c - nfull * gw    # leftover columns handled by DVE
            if gp > 0:
                pss = psum.tile([p, gp * gw], f32, tag="pss")
                for k in range(nfull):
                    c0 = k * gw
                    nc.tensor.matmul(
                        pss[:],
                        lhsT=ident_t[:],
                        rhs=e3[:, 0:gp, c0 : c0 + gw],
                        start=(k == 0),
                        stop=(k == nfull - 1),
                    )
                nc.vector.tensor_reduce(
                    out=s_sb[:, 0:gp],
                    in_=pss[:].rearrange("p (r g) -> p r g", r=gp),
                    axis=mybir.AxisListType.X,
                    op=Alu.add,
                )
                if crem > 0:
                    srem = small.tile([p, gp], f32, tag="srem")
                    nc.vector.tensor_reduce(
                        out=srem[:],
                        in_=e3[:, 0:gp, nfull * gw :],
                        axis=mybir.AxisListType.X,
                        op=Alu.add,
                    )
                    nc.vector.tensor_tensor(
                        out=s_sb[:, 0:gp], in0=s_sb[:, 0:gp], in1=srem[:],
                        op=Alu.add,
                    )
            if gp < r:
                nc.vector.tensor_reduce(
                    out=s_sb[:, gp:],
                    in_=e3[:, gp:r, :],
                    axis=mybir.AxisListType.X,
                    op=Alu.add,
                )

            # clamp away zeros (pad rows) then reciprocal
            nc.vector.tensor_scalar_max(s_sb[:], s_sb[:], 1e-30)
            rs = small.tile([p, r], f32, tag="rs")
            nc.vector.reciprocal(out=rs[:], in_=s_sb[:])

            # vals = [conf, acc, ones] laid out [p, 3, r]
            vals = small.tile([p, 3 * r], f32, tag="vals")
            v3 = vals[:].rearrange("p (k r) -> p k r", k=3)

            # me = float(pm & ~127); conf = me * (1/s)
            meb = small.tile([p, r], i32, tag="meb")
            nc.vector.scalar_tensor_tensor(
                out=meb[:], in0=pm[:].bitcast(i32), scalar=mask_hi[:],
                in1=zeros_i[:], op0=Alu.bitwise_and, op1=Alu.bitwise_or,
            )
            nc.vector.tensor_tensor(
                out=v3[:, 0, :], in0=meb[:].bitcast(f32), in1=rs[:], op=Alu.mult
            )
            # acc = (pm & 127) == (127 - label)
            jrev = small.tile([p, r], i32, tag="jrev")
            nc.vector.scalar_tensor_tensor(
                out=jrev[:], in0=pm[:].bitcast(i32), scalar=mask_lo[:],
                in1=zeros_i[:], op0=Alu.bitwise_and, op1=Alu.bitwise_or,
            )
            nc.vector.tensor_tensor(
                out=v3[:, 1, :], in0=jrev[:], in1=labt[:], op=Alu.is_equal
            )
            nc.vector.memset(v3[:, 2, :], 1.0)

            # ge[p, b, r] = conf > b/15   (strict: pad rows have conf == 0.0)
            ge = small.tile([p, NBINS * r], f32, tag="ge")
            g3 = ge[:].rearrange("p (b r) -> p b r", b=NBINS)
            for b in range(NBINS):
                nc.vector.tensor_single_scalar(
                    out=g3[:, b, :], in_=v3[:, 0, :], scalar=float(b) / NBINS,
                    op=Alu.is_gt,
                )

            # cumulative histogram: out[b, k] += sum_rows ge_b * vals_k.
            # Emitted one tile late so PE never stalls waiting for this
            # tile's DVE chain (keeps chunk-sum matmuls flowing).
            pend.append((g3, v3))
            if len(pend) > 1:
                _emit_hist(pend.pop(0))
        _emit_hist(pend.pop(0))

        nc.sync.dma_start(out=out[:, :], in_=hist[:])

    nc.finalize()
    return nc


# ---------------------------------------------------------------- host side

def _prep_core_inputs(logits, labels, core):
    """Build the per-core input dict (padded, tile-layout labels)."""
    lo = core * REAL_ROWS_PER_CORE
    hi = lo + REAL_ROWS_PER_CORE
    x = np.full((ROWS_PER_CORE, C), PAD_LOGIT, dtype=np.float32)
    x[: REAL_ROWS_PER_CORE] = logits[lo:hi]
    lab = np.zeros(ROWS_PER_CORE, dtype=np.int32)
    lab[: REAL_ROWS_PER_CORE] = labels[lo:hi].astype(np.int32)
    labrev = (127 - lab).reshape(T, P, R)
    return {"x": x, "lab": labrev}


def _shared_inputs():
    rev = np.broadcast_to(
        (127 - np.arange(C, dtype=np.int32))[None, None, :], (P, R, C)
    ).reshape(P, R * C).copy()
    ident = np.eye(P, dtype=np.float32)
    return {"rev": rev, "ident": ident}


def _finish(hists):
    """hists: list of [15, 3] cumulative-threshold sums -> (ece, mce)."""
    cum = np.zeros((NBINS + 1, 3), dtype=np.float64)
    for h in hists:
        cum[:NBINS] += h.astype(np.float64)
    per_bin = cum[:NBINS] - cum[1:]  # [15, 3]: sum_conf, sum_acc, count
    sum_conf, sum_acc, counts = per_bin[:, 0], per_bin[:, 1], per_bin[:, 2]
    nonempty = counts > 0
    safe = np.where(nonempty, counts, 1.0)
    gap = np.abs(sum_conf / safe - sum_acc / safe)
    n_total = float(2_000_000)
    ece = np.sum(np.where(nonempty, gap * counts / n_total, 0.0))
    mce = np.max(np.where(nonempty, gap, -np.inf)) if nonempty.any() else 1.0
    return np.float32(ece), np.float32(mce)


_NC_CACHE = {}


def kernel(logits, labels):
    from concourse.bass_utils import run_bass_kernel_spmd

    logits = np.asarray(logits, dtype=np.float32)
    labels = np.asarray(labels)

    if "nc" not in _NC_CACHE:
        _NC_CACHE["nc"] = build_nc()
    nc = _NC_CACHE["nc"]

    shared = _shared_inputs()
    in_maps = [
        {**_prep_core_inputs(logits, labels, core), **shared}
        for core in range(NCORES)
    ]
    res = run_bass_kernel_spmd(nc, in_maps, list(range(NCORES)))
    hists = [res.results[i]["out"] for i in range(NCORES)]
    return _finish(hists)
